# revision 6
# baseline (speedup 1.0000x reference)
"""2D Haar DWT (mode=0 'even') on Trainium2, 8 NeuronCores — bf16 I/O.

Input : x [2, 16, 16, 256, 256] f32, mode (0)
Output: [2, 64, 16, 128, 128] f32  (channel concat of LL, HL, LH, HH)

The kernel is HBM-bandwidth bound (in + out = 32 MiB/core in f32). The
correctness gate is an L2 relative-error bound of 2e-2, so the device
computes in bf16: the host folds the 0.5 prescale into an f32->bf16
conversion (L2 err ~2e-3), halving DMA traffic to 16 MiB/core
(~47 us at the 358 GB/s per-core HBM limit, vs ~94 us for f32).

Host-side layout: besides the dtype conversion, each 256-wide row is
stored column-deinterleaved as [128 even cols | 128 odd cols]. This
makes every DVE op innermost-stride-1, which is required for the bf16
tensor_tensor 2x perf mode (stride-2 operands drop to 1x and the DVE
would become the bottleneck).

Sharding: the 2*16 = 32 (b, c) pairs are split 4-per-core across 8
cores; no inter-core communication.

Per-core kernel (Tile framework), 4 iterations of 16 depth-images:
  - partition p = (d, q): image d in [0,16) x 32-row block q in [0,8),
    so each partition holds 32 consecutive input rows (16 KiB
    contiguous DRAM per input DMA) and produces 16 consecutive output
    rows per subband (4 KiB contiguous DRAM per output DMA).
  - input DMAs on the Sync HWDGE ring, output DMAs on the Scalar ring.
  - 4 DVE ops per chunk (all bf16 2x mode):
      vs = even_row + odd_row            vd = odd_row - even_row
      [LL|LH] = {vs,vd}_ecol + {vs,vd}_ocol   (one fused add)
      [HL|HH] = {vs,vd}_ocol - {vs,vd}_ecol   (one fused sub)
"""

import numpy as np

N_CORES = 8
B, C, D, H, W = 2, 16, 16, 256, 256
GROUPS_PER_CORE = 4  # (b,c) pairs per core
QB = 8               # 32-row blocks per image
RB = H // QB         # rows per partition block (32)

_compiled_nc = None


def _build_nc():
    import concourse.bacc as bacc
    import concourse.tile as tile
    import concourse.mybir as mybir

    bf16 = mybir.dt.bfloat16
    nc = bacc.Bacc("TRN2", target_bir_lowering=False, debug=False,
                   num_devices=N_CORES)

    # host delivers x column-deinterleaved: [g, d, h, par(2), w2(128)]
    x = nc.dram_tensor("x", [GROUPS_PER_CORE, D, H, 2, W // 2], bf16,
                       kind="ExternalInput")
    y = nc.dram_tensor("y", [GROUPS_PER_CORE, 4, D, H // 2, W // 2], bf16,
                       kind="ExternalOutput")

    # partition p = (d, q): image d (16), 32-row block q (8)
    # [4 iter, 128 part, 32 row, 256 w]; 16 KiB contiguous per partition
    xa = x.rearrange("g d (q r) par w -> g (d q) r (par w)", q=QB, r=RB)
    # output rows h2 = 16 q + e; 4 KiB contiguous per partition/subband
    ya = y.rearrange("g s d (q e) w -> g (d q) s e w", q=QB, e=RB // 2)

    with tile.TileContext(nc) as tc:
        with tc.tile_pool(name="io", bufs=3) as io_pool, \
             tc.tile_pool(name="mid", bufs=2) as mid_pool, \
             tc.tile_pool(name="outp", bufs=3) as out_pool:
            for g in range(GROUPS_PER_CORE):
                # small first chunks shrink the pipeline ramp; small last
                # chunks shrink the exposed drain after the final input
                if g == 0:
                    chunks = [(0, 4), (4, 12), (12, 20), (20, 32)]
                elif g == GROUPS_PER_CORE - 1:
                    chunks = [(0, 16), (16, 24), (24, 28), (28, 32)]
                else:
                    chunks = [(0, 16), (16, 32)]

                # g0: pre-issue every input DMA before any compute, on
                # alternating queues, so the ramp phase uses both HWDGE
                # rings (the scalar ring has no output work yet and its
                # input issues precede all output issues in program order)
                pre_tiles = {}
                if g == 0:
                    for ci, (r0, r1) in enumerate(chunks):
                        nr = r1 - r0
                        t = io_pool.tile([128, nr * W], bf16,
                                         tag=f"tin_g0_{ci}")
                        tv_ = t[:].rearrange("p (r w) -> p r w", r=nr)
                        eng = nc.sync if ci % 2 == 0 else nc.scalar
                        eng.dma_start(tv_, xa[g, :, r0:r1, :])
                        pre_tiles[ci] = t

                for ci, (r0, r1) in enumerate(chunks):
                    last = (g == GROUPS_PER_CORE - 1 and
                            ci == len(chunks) - 1)
                    nr = r1 - r0
                    ne = nr // 2
                    if ci in pre_tiles:
                        t_in = pre_tiles[ci]
                    else:
                        t_in = io_pool.tile([128, nr * W], bf16, tag="t_in")
                        t_in_v = t_in[:].rearrange("p (r w) -> p r w", r=nr)
                        nc.sync.dma_start(t_in_v, xa[g, :, r0:r1, :])

                    # rows r = 2e + ro; even/odd row views [128, ne, 256]
                    tv = t_in[:].rearrange("p (e ro pw) -> p e ro pw",
                                           e=ne, ro=2)
                    # vmid = [vs | vd]; vs = e+o rows, vd = o-e rows
                    vmid = mid_pool.tile([128, 2 * ne * W], bf16, tag="vmid")
                    vm = vmid[:].rearrange("p (m e pw) -> p m e pw",
                                           m=2, e=ne)
                    nc.vector.tensor_add(vm[:, 0], tv[:, :, 0, :],
                                         tv[:, :, 1, :])
                    nc.vector.tensor_sub(vm[:, 1], tv[:, :, 1, :],
                                         tv[:, :, 0, :])

                    # columns pre-split: pw = [par(2) w2(128)]
                    vc = vmid[:].rearrange(
                        "p (m e par w) -> p m e par w", m=2, e=ne, par=2)
                    # o = [a(2), m(2), e, w2]: a=0 adds -> LL,LH;
                    # a=1 subs -> HL,HH; subband s = 2m + a
                    o = out_pool.tile([128, 4 * ne * (W // 2)], bf16,
                                      tag="o")
                    ov = o[:].rearrange("p (a m e w) -> p a m e w",
                                        a=2, m=2, e=ne)
                    nc.vector.tensor_add(ov[:, 0], vc[:, :, :, 0, :],
                                         vc[:, :, :, 1, :])
                    nc.vector.tensor_sub(ov[:, 1], vc[:, :, :, 1, :],
                                         vc[:, :, :, 0, :])

                    e0, e1 = r0 // 2, r1 // 2
                    yc = ya[g, :, :, e0:e1, :]
                    # a=0 (adds) -> subbands {0, 2}; a=1 (subs) -> {1, 3}
                    # the final chunk drains on both HWDGE queues (the sync
                    # queue has no inputs left to block at that point)
                    nc.scalar.dma_start(yc[:, 0::2], ov[:, 0])
                    o1_eng = nc.sync if last else nc.scalar
                    o1_eng.dma_start(yc[:, 1::2], ov[:, 1])

    nc.compile()
    return nc


def _get_nc():
    global _compiled_nc
    if _compiled_nc is None:
        _compiled_nc = _build_nc()
    return _compiled_nc


def _haar_numpy(x):
    # mode='odd' fallback: pad one zero row/col at the end of H and W
    x = np.pad(x, ((0, 0), (0, 0), (0, 0), (0, 1), (0, 1)))
    x01 = x[:, :, :, 0::2, :] * 0.5
    x02 = x[:, :, :, 1::2, :] * 0.5
    x1 = x01[..., 0::2]
    x2 = x02[..., 0::2]
    x3 = x01[..., 1::2]
    x4 = x02[..., 1::2]
    return np.concatenate((x1 + x2 + x3 + x4, -x1 - x2 + x3 + x4,
                           -x1 + x2 - x3 + x4, x1 - x2 - x3 + x4), axis=1)


def _prep_input(x):
    """f32 [B,C,D,H,W] -> bf16 [B*C, D, H, 2, W/2] with 0.5 folded in and
    even/odd columns deinterleaved."""
    import ml_dtypes
    xr = np.asarray(x, dtype=np.float32).reshape(B * C, D, H, W // 2, 2)
    xq = (xr * np.float32(0.5)).astype(ml_dtypes.bfloat16)
    return np.ascontiguousarray(np.swapaxes(xq, -1, -2))


def _postprocess(out_bf16):
    """[32, 4, D, H/2, W/2] bf16 (core-major) -> [B, 4C, D, H/2, W/2] f32."""
    out = np.asarray(out_bf16).reshape(B, C, 4, D, H // 2, W // 2)
    out = out.transpose(0, 2, 1, 3, 4, 5).reshape(B, 4 * C, D,
                                                  H // 2, W // 2)
    return np.ascontiguousarray(out.astype(np.float32))


def run_device(in_maps, trace=False, **kwargs):
    """Run the compiled SPMD kernel; returns BassKernelResults."""
    from concourse.bass_utils import run_bass_kernel_spmd
    nc = _get_nc()
    return run_bass_kernel_spmd(nc, in_maps, core_ids=list(range(N_CORES)),
                                trace=trace, **kwargs)


_cached_exec = None  # (callable, out_shape) reused across kernel() calls


def _get_cached_exec():
    """Build the sharded PJRT executable once; jax caches its compilation
    across calls (run_bass_via_pjrt rebuilds the jit closure every call,
    paying retrace + XLA lowering each time)."""
    global _cached_exec
    if _cached_exec is not None:
        return _cached_exec
    import jax
    import ml_dtypes
    from jax.experimental.shard_map import shard_map
    from jax.sharding import Mesh, PartitionSpec
    from concourse import bass2jax

    bass2jax.install_neuronx_cc_hook()
    nc = _get_nc()
    out_shape = (GROUPS_PER_CORE, 4, D, H // 2, W // 2)
    out_aval = jax.core.ShapedArray(out_shape, ml_dtypes.bfloat16)

    def _body(x_arg, y_zero):
        outs = bass2jax._bass_exec_p.bind(
            x_arg, y_zero,
            out_avals=(out_aval,),
            in_names=("x", "y"),
            out_names=("y",),
            lowering_input_output_aliases=(),
            sim_require_finite=True,
            sim_require_nnan=True,
            nc=nc,
        )
        return (outs[0],)

    devices = jax.devices()[:N_CORES]
    mesh = Mesh(np.asarray(devices), ("core",))
    fn = jax.jit(
        shard_map(_body, mesh=mesh,
                  in_specs=(PartitionSpec("core"),) * 2,
                  out_specs=(PartitionSpec("core"),),
                  check_rep=False),
        donate_argnums=(1,), keep_unused=True)
    _cached_exec = (fn, out_shape)
    return _cached_exec


def make_in_maps(x):
    xs = _prep_input(x)
    return [{"x": xs[GROUPS_PER_CORE * k: GROUPS_PER_CORE * (k + 1)]}
            for k in range(N_CORES)]


def gather_output(results):
    out = np.stack([results[k]["y"] for k in range(N_CORES)])
    return _postprocess(out)


def _run_fast(x):
    import ml_dtypes
    fn, out_shape = _get_cached_exec()
    xs = _prep_input(x)
    zeros = np.zeros((N_CORES * out_shape[0], *out_shape[1:]),
                     ml_dtypes.bfloat16)
    (y,) = fn(xs, zeros)
    return _postprocess(y)


def kernel(x, mode):
    mode_val = int(np.asarray(mode))
    if mode_val != 0:
        return _haar_numpy(np.asarray(x, dtype=np.float32))
    try:
        return _run_fast(x)
    except Exception:
        pass  # fall back to the stock bass_utils path below
    in_maps = make_in_maps(x)
    try:
        res = run_device(in_maps)
    except Exception:
        res = run_device(in_maps)  # one retry for transient device errors
    return gather_output(res.results)


# revision 7
# speedup vs baseline: 1.0287x; 1.0287x over previous
"""2D Haar DWT (mode=0 'even') on Trainium2, 8 NeuronCores — bf16 I/O.

Input : x [2, 16, 16, 256, 256] f32, mode (0)
Output: [2, 64, 16, 128, 128] f32  (channel concat of LL, HL, LH, HH)

The kernel is HBM-bandwidth bound (in + out = 32 MiB/core in f32). The
correctness gate is an L2 relative-error bound of 2e-2, so the device
computes in bf16: the host folds the 0.5 prescale into an f32->bf16
conversion (L2 err ~2e-3), halving DMA traffic to 16 MiB/core
(~47 us at the 358 GB/s per-core HBM limit, vs ~94 us for f32).

Host-side layout: besides the dtype conversion, each 256-wide row is
stored column-deinterleaved as [128 even cols | 128 odd cols]. This
makes every DVE op innermost-stride-1, which is required for the bf16
tensor_tensor 2x perf mode (stride-2 operands drop to 1x and the DVE
would become the bottleneck).

Sharding: the 2*16 = 32 (b, c) pairs are split 4-per-core across 8
cores; no inter-core communication.

Per-core kernel (Tile framework), 4 iterations of 16 depth-images:
  - partition p = (d, q): image d in [0,16) x 32-row block q in [0,8),
    so each partition holds 32 consecutive input rows (16 KiB
    contiguous DRAM per input DMA) and produces 16 consecutive output
    rows per subband (4 KiB contiguous DRAM per output DMA).
  - input DMAs on the Sync HWDGE ring, output DMAs on the Scalar ring.
  - 4 DVE ops per chunk (all bf16 2x mode):
      vs = even_row + odd_row            vd = odd_row - even_row
      [LL|LH] = {vs,vd}_ecol + {vs,vd}_ocol   (one fused add)
      [HL|HH] = {vs,vd}_ocol - {vs,vd}_ecol   (one fused sub)
"""

import numpy as np

N_CORES = 8
B, C, D, H, W = 2, 16, 16, 256, 256
GROUPS_PER_CORE = 4  # (b,c) pairs per core
QB = 8               # 32-row blocks per image
RB = H // QB         # rows per partition block (32)

_compiled_nc = None


def _build_nc():
    import concourse.bacc as bacc
    import concourse.tile as tile
    import concourse.mybir as mybir

    bf16 = mybir.dt.bfloat16
    nc = bacc.Bacc("TRN2", target_bir_lowering=False, debug=False,
                   num_devices=N_CORES)

    # host delivers x column-deinterleaved: [g, d, h, par(2), w2(128)]
    x = nc.dram_tensor("x", [GROUPS_PER_CORE, D, H, 2, W // 2], bf16,
                       kind="ExternalInput")
    y = nc.dram_tensor("y", [GROUPS_PER_CORE, 4, D, H // 2, W // 2], bf16,
                       kind="ExternalOutput")

    # partition p = (d, q): image d (16), 32-row block q (8)
    # [4 iter, 128 part, 32 row, 256 w]; 16 KiB contiguous per partition
    xa = x.rearrange("g d (q r) par w -> g (d q) r (par w)", q=QB, r=RB)
    # output rows h2 = 16 q + e; 4 KiB contiguous per partition/subband
    ya = y.rearrange("g s d (q e) w -> g (d q) s e w", q=QB, e=RB // 2)

    with tile.TileContext(nc) as tc:
        with tc.tile_pool(name="io", bufs=3) as io_pool, \
             tc.tile_pool(name="mid", bufs=2) as mid_pool, \
             tc.tile_pool(name="outp", bufs=3) as out_pool:
            for g in range(GROUPS_PER_CORE):
                # small first chunks shrink the pipeline ramp; small last
                # chunks shrink the exposed drain after the final input
                if g == 0:
                    chunks = [(0, 4), (4, 12), (12, 20), (20, 32)]
                elif g == GROUPS_PER_CORE - 1:
                    chunks = [(0, 16), (16, 24), (24, 28), (28, 32)]
                else:
                    chunks = [(0, 16), (16, 32)]

                for ci, (r0, r1) in enumerate(chunks):
                    last = (g == GROUPS_PER_CORE - 1 and
                            ci == len(chunks) - 1)
                    nr = r1 - r0
                    ne = nr // 2
                    t_in = io_pool.tile([128, nr * W], bf16, tag="t_in")
                    t_in_v = t_in[:].rearrange("p (r w) -> p r w", r=nr)
                    nc.sync.dma_start(t_in_v, xa[g, :, r0:r1, :])

                    # rows r = 2e + ro; even/odd row views [128, ne, 256]
                    tv = t_in[:].rearrange("p (e ro pw) -> p e ro pw",
                                           e=ne, ro=2)
                    # vmid = [vs | vd]; vs = e+o rows, vd = o-e rows
                    vmid = mid_pool.tile([128, 2 * ne * W], bf16, tag="vmid")
                    vm = vmid[:].rearrange("p (m e pw) -> p m e pw",
                                           m=2, e=ne)
                    nc.vector.tensor_add(vm[:, 0], tv[:, :, 0, :],
                                         tv[:, :, 1, :])
                    nc.vector.tensor_sub(vm[:, 1], tv[:, :, 1, :],
                                         tv[:, :, 0, :])

                    # columns pre-split: pw = [par(2) w2(128)]
                    vc = vmid[:].rearrange(
                        "p (m e par w) -> p m e par w", m=2, e=ne, par=2)
                    # o = [a(2), m(2), e, w2]: a=0 adds -> LL,LH;
                    # a=1 subs -> HL,HH; subband s = 2m + a
                    o = out_pool.tile([128, 4 * ne * (W // 2)], bf16,
                                      tag="o")
                    ov = o[:].rearrange("p (a m e w) -> p a m e w",
                                        a=2, m=2, e=ne)
                    nc.vector.tensor_add(ov[:, 0], vc[:, :, :, 0, :],
                                         vc[:, :, :, 1, :])
                    nc.vector.tensor_sub(ov[:, 1], vc[:, :, :, 1, :],
                                         vc[:, :, :, 0, :])

                    e0, e1 = r0 // 2, r1 // 2
                    yc = ya[g, :, :, e0:e1, :]
                    # a=0 (adds) -> subbands {0, 2}; a=1 (subs) -> {1, 3}
                    # the final chunk drains on both HWDGE queues (the sync
                    # queue has no inputs left to block at that point)
                    nc.scalar.dma_start(yc[:, 0::2], ov[:, 0])
                    o1_eng = nc.sync if last else nc.scalar
                    o1_eng.dma_start(yc[:, 1::2], ov[:, 1])

    nc.compile()
    return nc


def _get_nc():
    global _compiled_nc
    if _compiled_nc is None:
        _compiled_nc = _build_nc()
    return _compiled_nc


def _haar_numpy(x):
    # mode='odd' fallback: pad one zero row/col at the end of H and W
    x = np.pad(x, ((0, 0), (0, 0), (0, 0), (0, 1), (0, 1)))
    x01 = x[:, :, :, 0::2, :] * 0.5
    x02 = x[:, :, :, 1::2, :] * 0.5
    x1 = x01[..., 0::2]
    x2 = x02[..., 0::2]
    x3 = x01[..., 1::2]
    x4 = x02[..., 1::2]
    return np.concatenate((x1 + x2 + x3 + x4, -x1 - x2 + x3 + x4,
                           -x1 + x2 - x3 + x4, x1 - x2 - x3 + x4), axis=1)


def _prep_input(x):
    """f32 [B,C,D,H,W] -> bf16 [B*C, D, H, 2, W/2] with 0.5 folded in and
    even/odd columns deinterleaved."""
    import ml_dtypes
    xr = np.asarray(x, dtype=np.float32).reshape(B * C, D, H, W // 2, 2)
    xq = (xr * np.float32(0.5)).astype(ml_dtypes.bfloat16)
    return np.ascontiguousarray(np.swapaxes(xq, -1, -2))


def _postprocess(out_bf16):
    """[32, 4, D, H/2, W/2] bf16 (core-major) -> [B, 4C, D, H/2, W/2] f32."""
    out = np.asarray(out_bf16).reshape(B, C, 4, D, H // 2, W // 2)
    out = out.transpose(0, 2, 1, 3, 4, 5).reshape(B, 4 * C, D,
                                                  H // 2, W // 2)
    return np.ascontiguousarray(out.astype(np.float32))


def run_device(in_maps, trace=False, **kwargs):
    """Run the compiled SPMD kernel; returns BassKernelResults."""
    from concourse.bass_utils import run_bass_kernel_spmd
    nc = _get_nc()
    return run_bass_kernel_spmd(nc, in_maps, core_ids=list(range(N_CORES)),
                                trace=trace, **kwargs)


_cached_exec = None  # (callable, out_shape) reused across kernel() calls


def _get_cached_exec():
    """Build the sharded PJRT executable once; jax caches its compilation
    across calls (run_bass_via_pjrt rebuilds the jit closure every call,
    paying retrace + XLA lowering each time)."""
    global _cached_exec
    if _cached_exec is not None:
        return _cached_exec
    import jax
    import ml_dtypes
    from jax.experimental.shard_map import shard_map
    from jax.sharding import Mesh, PartitionSpec
    from concourse import bass2jax

    bass2jax.install_neuronx_cc_hook()
    nc = _get_nc()
    out_shape = (GROUPS_PER_CORE, 4, D, H // 2, W // 2)
    out_aval = jax.core.ShapedArray(out_shape, ml_dtypes.bfloat16)

    def _body(x_arg, y_zero):
        outs = bass2jax._bass_exec_p.bind(
            x_arg, y_zero,
            out_avals=(out_aval,),
            in_names=("x", "y"),
            out_names=("y",),
            lowering_input_output_aliases=(),
            sim_require_finite=True,
            sim_require_nnan=True,
            nc=nc,
        )
        return (outs[0],)

    devices = jax.devices()[:N_CORES]
    mesh = Mesh(np.asarray(devices), ("core",))
    fn = jax.jit(
        shard_map(_body, mesh=mesh,
                  in_specs=(PartitionSpec("core"),) * 2,
                  out_specs=(PartitionSpec("core"),),
                  check_rep=False),
        donate_argnums=(1,), keep_unused=True)
    _cached_exec = (fn, out_shape)
    return _cached_exec


def make_in_maps(x):
    xs = _prep_input(x)
    return [{"x": xs[GROUPS_PER_CORE * k: GROUPS_PER_CORE * (k + 1)]}
            for k in range(N_CORES)]


def gather_output(results):
    out = np.stack([results[k]["y"] for k in range(N_CORES)])
    return _postprocess(out)


def _run_fast(x):
    import ml_dtypes
    fn, out_shape = _get_cached_exec()
    xs = _prep_input(x)
    zeros = np.zeros((N_CORES * out_shape[0], *out_shape[1:]),
                     ml_dtypes.bfloat16)
    (y,) = fn(xs, zeros)
    return _postprocess(y)


def kernel(x, mode):
    mode_val = int(np.asarray(mode))
    if mode_val != 0:
        return _haar_numpy(np.asarray(x, dtype=np.float32))
    try:
        return _run_fast(x)
    except Exception:
        pass  # fall back to the stock bass_utils path below
    in_maps = make_in_maps(x)
    try:
        res = run_device(in_maps)
    except Exception:
        res = run_device(in_maps)  # one retry for transient device errors
    return gather_output(res.results)
